# revision 1
# baseline (speedup 1.0000x reference)
"""Trainium2 Bass kernel for CrossModalAttention.

Reference computation (per (b, m) of B=4 x M=3):
    Q = x_q @ Wq.T + bq ; K = x_k @ Wk.T + bk ; V = x_v @ Wv.T + bv
    per head h (4 heads of dim 128):
        scores = Q_h @ K_h.T / sqrt(128)      [2048, 2048]
        attn   = softmax(scores, axis=-1)
        out_h  = attn @ V_h                   [2048, 128]

Sharding over 8 cores: 48 (b*m, head) units, 6 per core.
  core c: slot A = bm c      (all 4 heads)
          slot B = bm 8+c//2 (heads {0,1} if c even else {2,3})

On-device strategy per slot:
  - inputs are loaded pre-transposed (xT: contraction dim on partitions)
    straight from HBM via bf16 xbar DMA-transpose; one tile per 128-wide
    contraction slice so the first projection matmul only waits for the
    first transpose
  - QT, KT computed as [d, tok] (head dim on partitions), V as [tok, d]
  - scores are computed TRANSPOSED (ST[k, q] = K @ Q.T) so the attn @ V
    contraction over k uses V tiles as the stationary operand with no
    transposes of the [2048, 2048] attention matrix
  - no max-subtraction: scores are O(1) here, exp cannot overflow, and
    softmax is shift-invariant
  - softmax denominator: bf16 free-axis tree-sum over k-tiles on DVE,
    then PE-transpose of the remaining row so the partition-axis sum is
    a cheap free-axis reduce producing a per-q column; the division and
    the V-bias (out = attn@V_nobias / den + bv since sum(attn)=1) fold
    into the final psum->sbuf copy as one scalar_tensor_tensor
  - final out.T [d, q] chunks are transposed back via PE transpose
"""

import sys
import os

for _p in ("/root/.axon_site/_ro/trn_rl_repo", "/opt/trn_rl_repo"):
    if os.path.isdir(_p) and _p not in sys.path:
        sys.path.append(_p)

import numpy as np
import ml_dtypes

import concourse.bass as bass
import concourse.tile as tile
from concourse import bacc, mybir
from concourse.bass_utils import run_bass_kernel_spmd
from concourse.masks import make_identity

B, M, NTOK, DIM = 4, 3, 2048, 512
H, HD = 4, 128
NBM = B * M  # 12
NCORES = 8
SCALE = 1.0 / float(np.sqrt(HD))

F32 = mybir.dt.float32
BF16 = mybir.dt.bfloat16

TT = NTOK // 128  # 16 token tiles
CT = DIM // 128  # 4 contraction tiles
QCH = 512  # q is processed in chunks of 512
NQC = NTOK // QCH  # 4

# Knobs the test harness may flip before calling kernel():
TRACE = False
TRACE_KWARGS = {}
LAST_RESULTS = None

MULT = mybir.AluOpType.mult
ADD = mybir.AluOpType.add


def _emit_slot(nc, pools, dram, s, nh, ident, identb):
    """Emit instructions for one (bm, head-set) slot. nh = number of heads."""
    D = nh * HD
    (xtp, qkvp, wp, ep, accp, recp, outp, biasp, pst, ppv, ptp) = pools
    out_d = dram[f"out_{s}"]

    # ---- weights + biases up front (small; prefetch before transposes) ----
    ws = {}
    for wname in ("wq", "wk", "wv"):
        w = wp.tile([128, CT, D], BF16, tag=wname)
        nc.sync.dma_start(
            out=w[:, :, :],
            in_=dram[f"{wname}_{s}"][:].rearrange("(c p) d -> p c d", p=128),
        )
        ws[wname] = w
    # bq/bk laid out [p, which, head] so [*, i, dt:dt+1] is a per-partition
    # scalar for head dt; bv broadcast along partitions (added along free).
    bqk = biasp.tile([128, 2, nh], F32, tag="bqk")
    nc.sync.dma_start(
        out=bqk[:, 0, :], in_=dram[f"bq_{s}"][:].rearrange("(j p) -> p j", p=128)
    )
    nc.sync.dma_start(
        out=bqk[:, 1, :], in_=dram[f"bk_{s}"][:].rearrange("(j p) -> p j", p=128)
    )
    bvb = biasp.tile([128, D], F32, tag="bvb")
    nc.sync.dma_start(
        out=bvb[:, :], in_=dram[f"bv_{s}"][:].unsqueeze(0).to_broadcast([128, D])
    )

    # ---- projections ----
    QT = qkvp.tile([128, nh, NTOK], BF16, tag="qt")  # [d, head, tok]
    KT = qkvp.tile([128, nh, NTOK], BF16, tag="kt")
    V = qkvp.tile([128, TT, D], BF16, tag="v")  # [tok, ttile, d]

    def load_xt(xname):
        # per-ct tiles so each consumer matmul waits only on its own slice
        xr = dram[f"{xname}_{s}"][:].rearrange("M (c p) -> M c p", p=128)
        xts = []
        for ct in range(CT):
            xt = xtp.tile([128, NTOK], BF16, tag=f"xt{ct}")
            nc.sync.dma_start(out=xt[:, :], in_=xr[:, ct], transpose=True)
            xts.append(xt)
        return xts

    for which, (xname, wname, dst) in enumerate((("xq", "wq", QT), ("xk", "wk", KT))):
        xts = load_xt(xname)
        w = ws[wname]
        # dst[d, tok] = sum_c w[c, d] * xt[c, tok]  (+ bias[d])
        for dt in range(nh):
            for qc in range(NQC):
                ps = ppv.tile([128, QCH], F32, tag="pv")
                for ct in range(CT):
                    nc.tensor.matmul(
                        ps[:, :],
                        w[:, ct, dt * 128 : (dt + 1) * 128],
                        xts[ct][:, qc * QCH : (qc + 1) * QCH],
                        start=(ct == 0),
                        stop=(ct == CT - 1),
                    )
                nc.vector.tensor_scalar_add(
                    dst[:, dt, qc * QCH : (qc + 1) * QCH],
                    ps[:, :],
                    bqk[:, which, dt : dt + 1],
                )

    # V (no bias here: out = attn @ V / den + bv, since sum(attn) == 1)
    xts = load_xt("xv")
    w = ws["wv"]
    for tt in range(TT):
        ps = ppv.tile([128, D], F32, tag="pv")
        for ct in range(CT):
            nc.tensor.matmul(
                ps[:, :],
                xts[ct][:, tt * 128 : (tt + 1) * 128],
                w[:, ct, :],
                start=(ct == 0),
                stop=(ct == CT - 1),
            )
        nc.vector.tensor_copy(V[:, tt, :], ps[:, :])

    # ---- attention ----
    for h in range(nh):
        for qc in range(NQC):
            qsl = slice(qc * QCH, (qc + 1) * QCH)
            # E[k, q] = exp(scale * sum_d KT[d, k] QT[d, q]), k-tiled
            E = ep.tile([128, TT, QCH], BF16, tag="E")
            for g in range(TT // 2):
                st = pst.tile([128, 2 * QCH], F32, tag="st")
                for j in range(2):
                    kt = 2 * g + j
                    nc.tensor.matmul(
                        st[:, j * QCH : (j + 1) * QCH],
                        KT[:, h, kt * 128 : (kt + 1) * 128],
                        QT[:, h, qsl],
                        start=True,
                        stop=True,
                    )
                nc.scalar.activation(
                    E[:, 2 * g : 2 * g + 2, :],
                    st[:, :].rearrange("p (a b) -> p a b", b=QCH),
                    mybir.ActivationFunctionType.Exp,
                    scale=SCALE,
                )
            # denominator part 1: bf16 tree-sum over the 16 k-tiles
            # (free-axis adds; all-SBUF bf16 keeps the DVE 2x fast path)
            acc = accp.tile([128, 8, QCH], BF16, tag="acc")
            # the largest level runs on the otherwise-idle GPSIMD engine
            nc.gpsimd.tensor_add(acc[:, 0:8, :], E[:, 0:8, :], E[:, 8:16, :])
            nc.vector.tensor_add(acc[:, 0:4, :], acc[:, 0:4, :], acc[:, 4:8, :])
            nc.vector.tensor_add(acc[:, 0:2, :], acc[:, 0:2, :], acc[:, 2:4, :])
            nc.vector.tensor_add(acc[:, 0:1, :], acc[:, 0:1, :], acc[:, 1:2, :])

            # outT[d, q] = sum_k V[k, d] E[k, q]
            pv = ppv.tile([128, QCH], F32, tag="pv")
            for kt in range(TT):
                nc.tensor.matmul(
                    pv[:, :],
                    V[:, kt, h * 128 : (h + 1) * 128],
                    E[:, kt, :],
                    start=(kt == 0),
                    stop=(kt == TT - 1),
                )

            # denominator part 2: PE-transpose the summed row so the
            # partition-axis sum becomes a free-axis DVE reduce, giving the
            # denominator as a per-partition (per-q) column; reciprocal on
            # [128, 4] is ~20x cheaper than on [128, 512].
            dcol = recp.tile([128, NQC], F32, tag="dcol")
            for j in range(NQC):
                tpa = ptp.tile([128, 128], BF16, tag="tpa")
                nc.tensor.transpose(
                    tpa[:, :], acc[:, 0, j * 128 : (j + 1) * 128], identb[:, :]
                )
                nc.vector.reduce_sum(
                    out=dcol[:, j : j + 1], in_=tpa[:, :], axis=mybir.AxisListType.X
                )
            rec4 = recp.tile([128, NQC], F32, tag="rec4")
            nc.vector.reciprocal(rec4[:, :], dcol[:, :])

            outT = recp.tile([128, QCH], F32, tag="outT")
            nc.scalar.copy(outT[:, :], pv[:, :])

            # transpose back to [q, d]; the softmax division and the V bias
            # fold into the psum->sbuf copy: out = tp * (1/den) + bv
            ot = outp.tile([128, NQC, 128], F32, tag="ot")
            for j in range(NQC):
                tp = ptp.tile([128, 128], F32, tag="tp")
                nc.tensor.transpose(
                    tp[:, :], outT[:, j * 128 : (j + 1) * 128], ident[:, :]
                )
                nc.vector.scalar_tensor_tensor(
                    out=ot[:, j, :],
                    in0=tp[:, :],
                    scalar=rec4[:, j : j + 1],
                    in1=bvb[:, h * 128 : (h + 1) * 128],
                    op0=MULT,
                    op1=ADD,
                )
            nc.sync.dma_start(
                out=out_d[qc * QCH : (qc + 1) * QCH, h * 128 : (h + 1) * 128].rearrange(
                    "(j p) d -> p j d", p=128
                ),
                in_=ot[:, :, :],
            )


def _build_program():
    # Bacc (not plain Bass): its compile() pipeline legalizes multi-wait
    # instructions (walrus accepts at most 1 sync wait per instruction).
    nc = bacc.Bacc()
    dram = {}
    for s in ("a", "b"):
        D = 512 if s == "a" else 256
        for nm in ("xq", "xk", "xv"):
            dram[f"{nm}_{s}"] = nc.dram_tensor(
                f"{nm}_{s}", [NTOK, DIM], BF16, kind="ExternalInput"
            )
        for nm in ("wq", "wk", "wv"):
            dram[f"{nm}_{s}"] = nc.dram_tensor(
                f"{nm}_{s}", [DIM, D], BF16, kind="ExternalInput"
            )
        for nm in ("bq", "bk", "bv"):
            dram[f"{nm}_{s}"] = nc.dram_tensor(
                f"{nm}_{s}", [D], F32, kind="ExternalInput"
            )
        dram[f"out_{s}"] = nc.dram_tensor(
            f"out_{s}", [NTOK, D], F32, kind="ExternalOutput"
        )

    with tile.TileContext(nc) as tc:
        with (
            tc.tile_pool(name="singles", bufs=1) as singles,
            tc.tile_pool(name="xtp", bufs=2) as xtp,
            tc.tile_pool(name="qkvp", bufs=1) as qkvp,
            tc.tile_pool(name="wp", bufs=1) as wp,
            tc.tile_pool(name="ep", bufs=3) as ep,
            tc.tile_pool(name="accp", bufs=2) as accp,
            tc.tile_pool(name="recp", bufs=2) as recp,
            tc.tile_pool(name="outp", bufs=3) as outp,
            tc.tile_pool(name="biasp", bufs=1) as biasp,
            tc.tile_pool(name="pst", bufs=2, space="PSUM") as pst,
            tc.tile_pool(name="ppv", bufs=2, space="PSUM") as ppv,
            tc.tile_pool(name="ptp", bufs=1, space="PSUM") as ptp,
        ):
            ident = singles.tile([128, 128], F32, tag="ident")
            make_identity(nc, ident[:, :])
            identb = singles.tile([128, 128], BF16, tag="identb")
            make_identity(nc, identb[:, :])

            pools = (xtp, qkvp, wp, ep, accp, recp, outp, biasp, pst, ppv, ptp)
            _emit_slot(nc, pools, dram, "a", 4, ident, identb)
            _emit_slot(nc, pools, dram, "b", 2, ident, identb)

    # Run Bacc's compile pipeline (register allocation, sync-wait
    # legalization, nop fusion) — run_bass_via_pjrt does not call it.
    nc.finalize()
    return nc


_PROGRAM = None


def _get_program():
    global _PROGRAM
    if _PROGRAM is None:
        _PROGRAM = _build_program()
    return _PROGRAM


def kernel(query, key, value, Wq, bq, Wk, bk, Wv, bv):
    global LAST_RESULTS
    bf = ml_dtypes.bfloat16
    q = np.ascontiguousarray(np.asarray(query, np.float32).reshape(NBM, NTOK, DIM)).astype(bf)
    k = np.ascontiguousarray(np.asarray(key, np.float32).reshape(NBM, NTOK, DIM)).astype(bf)
    v = np.ascontiguousarray(np.asarray(value, np.float32).reshape(NBM, NTOK, DIM)).astype(bf)
    WqT = np.ascontiguousarray(np.asarray(Wq, np.float32).T).astype(bf)
    WkT = np.ascontiguousarray(np.asarray(Wk, np.float32).T).astype(bf)
    WvT = np.ascontiguousarray(np.asarray(Wv, np.float32).T).astype(bf)
    bq = np.asarray(bq, np.float32)
    bk = np.asarray(bk, np.float32)
    bv = np.asarray(bv, np.float32)

    in_maps = []
    for c in range(NCORES):
        bm_a = c
        bm_b = 8 + c // 2
        hs = (c % 2) * 256  # head-pair column offset for slot B
        in_maps.append(
            {
                "xq_a": q[bm_a], "xk_a": k[bm_a], "xv_a": v[bm_a],
                "xq_b": q[bm_b], "xk_b": k[bm_b], "xv_b": v[bm_b],
                "wq_a": WqT, "wk_a": WkT, "wv_a": WvT,
                "bq_a": bq, "bk_a": bk, "bv_a": bv,
                "wq_b": np.ascontiguousarray(WqT[:, hs : hs + 256]),
                "wk_b": np.ascontiguousarray(WkT[:, hs : hs + 256]),
                "wv_b": np.ascontiguousarray(WvT[:, hs : hs + 256]),
                "bq_b": np.ascontiguousarray(bq[hs : hs + 256]),
                "bk_b": np.ascontiguousarray(bk[hs : hs + 256]),
                "bv_b": np.ascontiguousarray(bv[hs : hs + 256]),
            }
        )

    nc = _get_program()
    res = run_bass_kernel_spmd(
        nc, in_maps, list(range(NCORES)), trace=TRACE, **TRACE_KWARGS
    )
    LAST_RESULTS = res

    out = np.empty((NBM, NTOK, DIM), np.float32)
    for c in range(NCORES):
        hs = (c % 2) * 256
        out[c] = res.results[c]["out_a"]
        out[8 + c // 2][:, hs : hs + 256] = res.results[c]["out_b"]
    return out.reshape(B, M, NTOK, DIM)



# revision 16
# speedup vs baseline: 1.0064x; 1.0064x over previous
"""Trainium2 Bass kernel for CrossModalAttention (v2, software-pipelined).

Reference computation (per (b, m) of B=4 x M=3):
    Q = x_q @ Wq.T + bq ; K = x_k @ Wk.T + bk ; V = x_v @ Wv.T + bv
    per head h (4 heads of dim 128):
        scores = Q_h @ K_h.T / sqrt(128)      [2048, 2048]
        attn   = softmax(scores, axis=-1)
        out_h  = attn @ V_h                   [2048, 128]

Sharding over 8 cores: 48 (b*m, head) units, 6 per core.
  core c: slot A = bm c      (all 4 heads)
          slot B = bm 8+c//2 (heads {0,1} if c even else {2,3})

v2 design notes (vs the v1 baseline at ~450us traced):
  - PE runs ONLY matmuls: all 128x128 transposes (softmax-denominator
    column extraction + output [d,q]->[q,d]) moved to the DMA xbar
    transpose engine; PE never stalls on slow DVE/gpsimd producers.
  - softmax-denominator tree-sum over the 16 k-tiles is column-split
    across GPSIMD (cols 0:128) and DVE (cols 128:512) so both engines
    reduce in parallel (the v1 monolithic gpsimd add took 7.9us/unit).
  - the whole kernel is emitted as one software-pipelined stream of 24
    "units" (head, q-chunk): unit u's scores MMs interleave with unit
    u-1's attn@V MMs and unit u-1's tail (transposes, divide, store);
    QKV projection matmuls are injected into the stream with an
    earliest-deadline schedule so PE load is ~even across steps.
  - input DMA-transposes are split across both HWDGE rings (sync +
    scalar) so x_q/x_k and x_v load in parallel at kernel start.
  - output is stored bf16 (host upcasts); softmax divide + bv bias fold
    into one scalar_tensor_tensor on the transposed output.
"""

import sys
import os

for _p in ("/root/.axon_site/_ro/trn_rl_repo", "/opt/trn_rl_repo"):
    if os.path.isdir(_p) and _p not in sys.path:
        sys.path.append(_p)

import numpy as np
import ml_dtypes

import concourse.bass as bass
import concourse.tile as tile
from concourse import bacc, mybir
from concourse.bass_utils import run_bass_kernel_spmd

B, M, NTOK, DIM = 4, 3, 2048, 512
H, HD = 4, 128
NBM = B * M  # 12
NCORES = 8
SCALE = 1.0 / float(np.sqrt(HD))

F32 = mybir.dt.float32
BF16 = mybir.dt.bfloat16

TT = NTOK // 128  # 16 k tiles
CT = DIM // 128  # 4 contraction tiles
QCH = 512  # q processed in chunks of 512
NQC = NTOK // QCH  # 4
GSPLIT = 128  # tree-sum column split: gpsimd takes [0:128), DVE [128:512)

# Knobs the test harness may flip before calling kernel():
TRACE = False
TRACE_KWARGS = {}
LAST_RESULTS = None

MULT = mybir.AluOpType.mult
ADD = mybir.AluOpType.add
EXP = mybir.ActivationFunctionType.Exp
AXX = mybir.AxisListType.X


def _build_program():
    nc = bacc.Bacc()
    dram = {}
    for s in ("a", "b"):
        D = 512 if s == "a" else 256
        # x inputs arrive PRE-TRANSPOSED from the host ([DIM, NTOK]) so they
        # load with plain DMAs — the xbar transpose unit is reserved for the
        # per-unit SBUF->SBUF transposes (concurrent xbar use from two HWDGE
        # rings corrupts data; see minitest3).
        for nm in ("xq", "xk", "xv"):
            dram[f"{nm}_{s}"] = nc.dram_tensor(
                f"{nm}_{s}", [DIM, NTOK], BF16, kind="ExternalInput"
            )
        for nm in ("wq", "wk", "wv"):
            dram[f"{nm}_{s}"] = nc.dram_tensor(
                f"{nm}_{s}", [DIM, D], BF16, kind="ExternalInput"
            )
        for nm in ("bq", "bk"):
            dram[f"{nm}_{s}"] = nc.dram_tensor(
                f"{nm}_{s}", [D], F32, kind="ExternalInput"
            )
        dram[f"bv_{s}"] = nc.dram_tensor(f"bv_{s}", [D], BF16, kind="ExternalInput")
        dram[f"out_{s}"] = nc.dram_tensor(
            f"out_{s}", [NTOK, D], BF16, kind="ExternalOutput"
        )

    with tile.TileContext(nc) as tc:
        with (
            tc.tile_pool(name="wp", bufs=1) as wp,
            tc.tile_pool(name="biasp", bufs=1) as biasp,
            tc.tile_pool(name="xtp", bufs=1) as xtp,
            tc.tile_pool(name="qkvp", bufs=1) as qkvp,
            tc.tile_pool(name="ep", bufs=2) as ep,
            tc.tile_pool(name="accp", bufs=3) as accp,
            tc.tile_pool(name="accTp", bufs=2) as accTp,
            tc.tile_pool(name="outTp", bufs=2) as outTp,
            tc.tile_pool(name="otqp", bufs=2) as otqp,
            tc.tile_pool(name="otp", bufs=2) as otp,
            tc.tile_pool(name="recp", bufs=2) as recp,
            tc.tile_pool(name="pst", bufs=2, space="PSUM") as pst,
            tc.tile_pool(name="ppv", bufs=2, space="PSUM") as ppv,
        ):
            _emit(nc, dram, wp, biasp, xtp, qkvp, ep, accp, accTp, outTp,
                  otqp, otp, recp, pst, ppv)

    nc.finalize()
    return nc


def _emit(nc, dram, wp, biasp, xtp, qkvp, ep, accp, accTp, outTp, otqp, otp,
          recp, pst, ppv):
    # ---------------- prologue: weights / biases / slot-A inputs ----------
    # scalar (ACT) HWDGE ring: wq, wk, then xq/xk transposes
    # sync (SP) HWDGE ring: wv, xv transposes, biases, slot-B weights
    ws = {}
    for s, D in (("a", 512), ("b", 256)):
        for wname, eng in (("wq", nc.scalar), ("wk", nc.scalar), ("wv", nc.sync)):
            w = wp.tile([128, CT, D], BF16, tag=f"{wname}_{s}")
            eng.dma_start(
                out=w[:, :, :],
                in_=dram[f"{wname}_{s}"][:].rearrange("(c p) d -> p c d", p=128),
            )
            ws[f"{wname}_{s}"] = w

    xts = {}  # (slot, kind, ct) -> tile ; tags shared between slots (serial reuse)

    def load_x(s, kind, eng):
        xr = dram[f"x{kind}_{s}"][:].rearrange("(c p) M -> p c M", p=128)
        for ct in range(CT):
            xt = xtp.tile([128, NTOK], BF16, tag=f"x{kind}{ct}")
            eng.dma_start(out=xt[:, :], in_=xr[:, ct])
            xts[(s, kind, ct)] = xt

    load_x("a", "q", nc.scalar)
    load_x("a", "k", nc.scalar)
    load_x("a", "v", nc.sync)

    bqk = {}
    bvb = {}
    for s, D in (("a", 512), ("b", 256)):
        nh = D // 128
        t = biasp.tile([128, 2, nh], F32, tag=f"bqk_{s}")
        nc.sync.dma_start(
            out=t[:, 0, :], in_=dram[f"bq_{s}"][:].rearrange("(j p) -> p j", p=128)
        )
        nc.sync.dma_start(
            out=t[:, 1, :], in_=dram[f"bk_{s}"][:].rearrange("(j p) -> p j", p=128)
        )
        bqk[s] = t
        bv = biasp.tile([128, D], BF16, tag=f"bvb_{s}")
        nc.sync.dma_start(
            out=bv[:, :], in_=dram[f"bv_{s}"][:].unsqueeze(0).to_broadcast([128, D])
        )
        bvb[s] = bv

    # persistent QKV tiles (distinct tags per slot so slot B projections can
    # overlap slot A attention)
    QT = {"a": qkvp.tile([128, 4, NTOK], BF16, tag="qt_a", name="qt_a"),
          "b": qkvp.tile([128, 2, NTOK], BF16, tag="qt_b", name="qt_b")}
    KT = {"a": qkvp.tile([128, 4, NTOK], BF16, tag="kt_a", name="kt_a"),
          "b": qkvp.tile([128, 2, NTOK], BF16, tag="kt_b", name="kt_b")}
    V = {"a": qkvp.tile([128, TT, 512], BF16, tag="v_a", name="v_a"),
         "b": qkvp.tile([128, TT, 256], BF16, tag="v_b", name="v_b")}

    # ------------- projection sub-task closures (injected into stream) ----
    def qk_sub(s, which, wname, dt, qc4):
        # 4 MMs (ct) + 1 bias-add: one (head, q-chunk-of-512) strip of Q or K
        def emit():
            dst = QT[s] if which == 0 else KT[s]
            w = ws[f"{wname}_{s}"]
            kind = "q" if which == 0 else "k"
            ps = ppv.tile([128, 512], F32, tag="psproj")
            for ct in range(CT):
                nc.tensor.matmul(
                    ps[:, :],
                    w[:, ct, dt * 128 : (dt + 1) * 128],
                    xts[(s, kind, ct)][:, qc4 * 512 : (qc4 + 1) * 512],
                    start=(ct == 0),
                    stop=(ct == CT - 1),
                )
            nc.vector.tensor_scalar_add(
                dst[:, dt, qc4 * 512 : (qc4 + 1) * 512],
                ps[:, :],
                bqk[s][:, which, dt : dt + 1],
            )
        return emit

    def v_sub(s, D, tt):
        # 4 MMs (ct) + 1 copy: one token-tile row strip of V
        def emit():
            ps = ppv.tile([128, 512], F32, tag="psproj")
            for ct in range(CT):
                nc.tensor.matmul(
                    ps[:, :D],
                    xts[(s, "v", ct)][:, tt * 128 : (tt + 1) * 128],
                    ws[f"wv_{s}"][:, ct, :],
                    start=(ct == 0),
                    stop=(ct == CT - 1),
                )
            nc.vector.tensor_copy(V[s][:, tt, :], ps[:, :D])
        return emit

    def load_sub(s, kind, eng):
        def emit():
            load_x(s, kind, eng)
        return emit

    # slot A h0 Q/K emitted in prologue (before step 0)
    for qc4 in range(NQC):
        qk_sub("a", 0, "wq", 0, qc4)()
        qk_sub("a", 1, "wk", 0, qc4)()

    # injection schedule: step -> list of closures (each ~4 MMs)
    inject = {st: [] for st in range(26)}

    def add_qk(step0, s, h):
        subs = [qk_sub(s, w, ("wq", "wk")[w], h, qc4)
                for w in range(2) for qc4 in range(NQC)]
        for i, sub in enumerate(subs):  # 8 subs over 2 steps
            inject[step0 + i // 4].append(sub)

    def add_v(step0, s, D, nsteps):
        subs = [v_sub(s, D, tt) for tt in range(TT)]
        per = (len(subs) + nsteps - 1) // nsteps
        for i, sub in enumerate(subs):
            inject[step0 + i // per].append(sub)

    add_v(0, "a", 512, 2)        # V_a: steps 0-1 (AV_0 runs in step 1)
    add_qk(2, "a", 1)            # h1 by end of step 3 (used step 4)
    add_qk(4, "a", 2)            # h2 by end of step 5 (used step 8)
    add_qk(6, "a", 3)            # h3 by end of step 7 (used step 12)
    # slot-B input loads: emitted at step 8 (slot-A x tags all free after
    # the h3 projections of step 7; xv_a free after step 1)
    inject[8].append(load_sub("b", "q", nc.scalar))
    inject[8].append(load_sub("b", "k", nc.scalar))
    inject[8].append(load_sub("b", "v", nc.sync))
    add_qk(10, "b", 0)           # by end of step 11 (used step 16)
    add_v(12, "b", 256, 3)       # steps 12-14 (AV_16 runs in step 17)
    add_qk(15, "b", 1)           # by end of step 16 (used step 20)

    # ---------------- the 24-unit software-pipelined stream ---------------
    units = [("a", h, qc) for h in range(4) for qc in range(NQC)] + \
            [("b", h, qc) for h in range(2) for qc in range(NQC)]

    state = {}  # unit idx -> dict of live tiles

    def emit_scores_g(u, g, st_tile):
        s, h, qc = units[u]
        qsl = slice(qc * QCH, (qc + 1) * QCH)
        for j in range(2):
            kt = 2 * g + j
            nc.tensor.matmul(
                st_tile[:, j * QCH : (j + 1) * QCH],
                KT[s][:, h, kt * 128 : (kt + 1) * 128],
                QT[s][:, h, qsl],
                start=True,
                stop=True,
            )

    def emit_av_g(u, g):
        s, h, qc = units[u]
        stt = state[u]
        for j in range(2):
            kt = 2 * g + j
            nc.tensor.matmul(
                stt["pv"][:, :],
                V[s][:, kt, h * 128 : (h + 1) * 128],
                stt["E"][:, kt, :],
                start=(kt == 0),
                stop=(kt == TT - 1),
            )

    NSTEPS = len(units) + 2
    for step in range(NSTEPS):
        u_cur = step if step < len(units) else None
        u_prev = step - 1 if 1 <= step <= len(units) else None
        u_tail = step - 2 if step >= 2 else None

        if u_cur is not None:
            E = ep.tile([128, TT, QCH], BF16, tag="E")
            state[u_cur] = {"E": E}
        if u_prev is not None:
            pv = ppv.tile([128, QCH], F32, tag="pvav")
            state[u_prev]["pv"] = pv

        inj = list(inject.get(step, ()))
        ninj = len(inj)

        # interleaved PE stream: projections | attn@V(u-1) | scores(u).
        # Projection subs MUST precede the AV chunk: in step 1 the g=7 AV
        # matmul reads V[:,15,:] which the g=7 injected V-sub writes.
        for g in range(8):
            # spread injected projection subs across the 8 g-slots
            lo = (g * ninj) // 8
            hi = ((g + 1) * ninj) // 8
            for sub in inj[lo:hi]:
                sub()
            if u_prev is not None:
                emit_av_g(u_prev, g)
            if u_cur is not None:
                st_tile = pst.tile([128, 2 * QCH], F32, tag="st")
                emit_scores_g(u_cur, g, st_tile)
            if u_cur is not None:
                nc.scalar.activation(
                    state[u_cur]["E"][:, 2 * g : 2 * g + 2, :],
                    st_tile[:, :].rearrange("p (a b) -> p a b", b=QCH),
                    EXP,
                    scale=SCALE,
                )

        # tail of unit u-2: denominator column extraction, out transpose,
        # divide+bias, store.  All deps completed during the previous step.
        if u_tail is not None:
            ts, th, tqc = units[u_tail]
            stt = state[u_tail]
            accT = accTp.tile([128, NQC, 128], BF16, tag="accT")
            nc.sync.dma_start_transpose(out=accT[:, 0, :], in_=stt["acc_g"][:, 0, :])
            nc.sync.dma_start_transpose(out=accT[:, 1:4, :], in_=stt["acc_v"][:, 0, :])
            otq = otqp.tile([128, NQC, 128], BF16, tag="otq")
            nc.sync.dma_start_transpose(out=otq[:, :, :], in_=stt["outT"][:, :])
            dcol4 = recp.tile([128, NQC], F32, tag="dcol4")
            rec4 = recp.tile([128, NQC], F32, tag="rec4")
            nc.vector.reduce_sum(out=dcol4[:, :], in_=accT[:, :, :], axis=AXX)
            nc.vector.reciprocal(rec4[:, :], dcol4[:, :])
            ot = otp.tile([128, NQC, 128], BF16, tag="ot")
            for j in range(NQC):
                nc.vector.scalar_tensor_tensor(
                    out=ot[:, j, :],
                    in0=otq[:, j, :],
                    scalar=rec4[:, j : j + 1],
                    in1=bvb[ts][:, th * 128 : (th + 1) * 128],
                    op0=MULT,
                    op1=ADD,
                )
            nc.sync.dma_start(
                out=dram[f"out_{ts}"][
                    tqc * QCH : (tqc + 1) * QCH, th * 128 : (th + 1) * 128
                ].rearrange("(j p) d -> p j d", p=128),
                in_=ot[:, :, :],
            )
            del state[u_tail]

        # cast pv(u-1) -> bf16 SBUF so the xbar can transpose it next step
        if u_prev is not None:
            outT = outTp.tile([128, QCH], BF16, tag="outT")
            nc.vector.tensor_copy(outT[:, :], state[u_prev]["pv"][:, :])
            state[u_prev]["outT"] = outT

        # tree-sum of unit u: gpsimd cols [0:GSPLIT), DVE cols [GSPLIT:512)
        if u_cur is not None:
            E = state[u_cur]["E"]
            acc_g = accp.tile([128, 8, GSPLIT], BF16, tag="acc_g")
            acc_v = accp.tile([128, 8, QCH - GSPLIT], BF16, tag="acc_v")
            nc.gpsimd.tensor_add(
                acc_g[:, :, :], E[:, 0:8, 0:GSPLIT], E[:, 8:16, 0:GSPLIT]
            )
            nc.gpsimd.tensor_add(
                acc_g[:, 0:4, :], acc_g[:, 0:4, :], acc_g[:, 4:8, :]
            )
            nc.gpsimd.tensor_add(
                acc_g[:, 0:2, :], acc_g[:, 0:2, :], acc_g[:, 2:4, :]
            )
            nc.gpsimd.tensor_add(
                acc_g[:, 0:1, :], acc_g[:, 0:1, :], acc_g[:, 1:2, :]
            )
            nc.vector.tensor_add(
                acc_v[:, :, :], E[:, 0:8, GSPLIT:QCH], E[:, 8:16, GSPLIT:QCH]
            )
            nc.vector.tensor_add(
                acc_v[:, 0:4, :], acc_v[:, 0:4, :], acc_v[:, 4:8, :]
            )
            nc.vector.tensor_add(
                acc_v[:, 0:2, :], acc_v[:, 0:2, :], acc_v[:, 2:4, :]
            )
            nc.vector.tensor_add(
                acc_v[:, 0:1, :], acc_v[:, 0:1, :], acc_v[:, 1:2, :]
            )
            state[u_cur]["acc_g"] = acc_g
            state[u_cur]["acc_v"] = acc_v


_PROGRAM = None


def _get_program():
    global _PROGRAM
    if _PROGRAM is None:
        _PROGRAM = _build_program()
    return _PROGRAM


def kernel(query, key, value, Wq, bq, Wk, bk, Wv, bv):
    global LAST_RESULTS
    bf = ml_dtypes.bfloat16
    q = np.ascontiguousarray(
        np.asarray(query, np.float32).reshape(NBM, NTOK, DIM).transpose(0, 2, 1)
    ).astype(bf)
    k = np.ascontiguousarray(
        np.asarray(key, np.float32).reshape(NBM, NTOK, DIM).transpose(0, 2, 1)
    ).astype(bf)
    v = np.ascontiguousarray(
        np.asarray(value, np.float32).reshape(NBM, NTOK, DIM).transpose(0, 2, 1)
    ).astype(bf)
    WqT = np.ascontiguousarray(np.asarray(Wq, np.float32).T).astype(bf)
    WkT = np.ascontiguousarray(np.asarray(Wk, np.float32).T).astype(bf)
    WvT = np.ascontiguousarray(np.asarray(Wv, np.float32).T).astype(bf)
    bq = np.asarray(bq, np.float32)
    bk = np.asarray(bk, np.float32)
    bvb = np.asarray(bv, np.float32).astype(bf)

    in_maps = []
    for c in range(NCORES):
        bm_a = c
        bm_b = 8 + c // 2
        hs = (c % 2) * 256  # head-pair column offset for slot B
        in_maps.append(
            {
                "xq_a": q[bm_a], "xk_a": k[bm_a], "xv_a": v[bm_a],
                "xq_b": q[bm_b], "xk_b": k[bm_b], "xv_b": v[bm_b],
                "wq_a": WqT, "wk_a": WkT, "wv_a": WvT,
                "bq_a": bq, "bk_a": bk, "bv_a": bvb,
                "wq_b": np.ascontiguousarray(WqT[:, hs : hs + 256]),
                "wk_b": np.ascontiguousarray(WkT[:, hs : hs + 256]),
                "wv_b": np.ascontiguousarray(WvT[:, hs : hs + 256]),
                "bq_b": np.ascontiguousarray(bq[hs : hs + 256]),
                "bk_b": np.ascontiguousarray(bk[hs : hs + 256]),
                "bv_b": np.ascontiguousarray(bvb[hs : hs + 256]),
            }
        )

    nc = _get_program()
    res = run_bass_kernel_spmd(
        nc, in_maps, list(range(NCORES)), trace=TRACE, **TRACE_KWARGS
    )
    LAST_RESULTS = res

    out = np.empty((NBM, NTOK, DIM), np.float32)
    for c in range(NCORES):
        hs = (c % 2) * 256
        out[c] = np.asarray(res.results[c]["out_a"], np.float32)
        out[8 + c // 2][:, hs : hs + 256] = np.asarray(
            res.results[c]["out_b"], np.float32
        )
    return out.reshape(B, M, NTOK, DIM)


# revision 20
# speedup vs baseline: 1.1539x; 1.1466x over previous
"""Trainium2 Bass kernel for CrossModalAttention (v4, software-pipelined).

Reference computation (per (b, m) of B=4 x M=3):
    Q = x_q @ Wq.T + bq ; K = x_k @ Wk.T + bk ; V = x_v @ Wv.T + bv
    per head h (4 heads of dim 128):
        scores = Q_h @ K_h.T / sqrt(128)      [2048, 2048]
        attn   = softmax(scores, axis=-1)
        out_h  = attn @ V_h                   [2048, 128]

Sharding over 8 cores: 48 (b*m, head) units, 6 per core.
  core c: slot A = bm c      (all 4 heads)
          slot B = bm 8+c//2 (heads {0,1} if c even else {2,3})

v4 design notes:
  - PE runs ONLY matmuls; the two per-unit 128x512 transposes (softmax
    denominator columns, output [d,q]->[q,d]) are single-call xbar DMA
    transposes with 3D out APs, all serialized on the sync HWDGE ring
    (concurrent xbar use from two rings corrupts data - minitest3).
  - x inputs arrive host-pre-transposed; all loads are plain DMAs split
    across the scalar + sync rings.
  - E and the tree-sum accumulators are FLAT 2D tiles; the denominator
    tree is one contiguous gpsimd add (L1 of k-tiles 0:8) in parallel
    with DVE adds (k-tiles 8:16 + merge levels).
  - every projection sub-task's PSUM->SBUF consumer (bias-add / V copy)
    is emitted one injection slot AFTER its matmuls so the DVE FIFO
    never head-of-line blocks on un-run PE work.
  - 24-unit software pipeline: unit u emits scores at step u, attn@V at
    step u+1, tail (transposes, divide+bias, store) at step u+2.
"""

import sys
import os

for _p in ("/root/.axon_site/_ro/trn_rl_repo", "/opt/trn_rl_repo"):
    if os.path.isdir(_p) and _p not in sys.path:
        sys.path.append(_p)

import numpy as np
import ml_dtypes

import concourse.bass as bass
import concourse.tile as tile
from concourse import bacc, mybir
from concourse.bass_utils import run_bass_kernel_spmd

B, M, NTOK, DIM = 4, 3, 2048, 512
H, HD = 4, 128
NBM = B * M  # 12
NCORES = 8
SCALE = 1.0 / float(np.sqrt(HD))

F32 = mybir.dt.float32
BF16 = mybir.dt.bfloat16

TT = NTOK // 128  # 16 k tiles
CT = DIM // 128  # 4 contraction tiles
QCH = 512  # q processed in chunks of 512
NQC = NTOK // QCH  # 4

# Knobs the test harness may flip before calling kernel():
TRACE = False
TRACE_KWARGS = {}
LAST_RESULTS = None

MULT = mybir.AluOpType.mult
ADD = mybir.AluOpType.add
EXP = mybir.ActivationFunctionType.Exp
AXX = mybir.AxisListType.X


def _build_program():
    nc = bacc.Bacc()
    dram = {}
    for s in ("a", "b"):
        D = 512 if s == "a" else 256
        for nm in ("xq", "xk", "xv"):
            dram[f"{nm}_{s}"] = nc.dram_tensor(
                f"{nm}_{s}", [DIM, NTOK], BF16, kind="ExternalInput"
            )
        for nm in ("wq", "wk", "wv"):
            dram[f"{nm}_{s}"] = nc.dram_tensor(
                f"{nm}_{s}", [DIM, D], BF16, kind="ExternalInput"
            )
        for nm in ("bq", "bk"):
            dram[f"{nm}_{s}"] = nc.dram_tensor(
                f"{nm}_{s}", [D], F32, kind="ExternalInput"
            )
        dram[f"bv_{s}"] = nc.dram_tensor(f"bv_{s}", [D], BF16, kind="ExternalInput")
        dram[f"out_{s}"] = nc.dram_tensor(
            f"out_{s}", [NTOK, D], BF16, kind="ExternalOutput"
        )

    with tile.TileContext(nc) as tc:
        with (
            tc.tile_pool(name="wp", bufs=1) as wp,
            tc.tile_pool(name="biasp", bufs=1) as biasp,
            tc.tile_pool(name="xtp", bufs=1) as xtp,
            tc.tile_pool(name="qkvp", bufs=1) as qkvp,
            tc.tile_pool(name="ep", bufs=2) as ep,
            tc.tile_pool(name="accp", bufs=3) as accp,
            tc.tile_pool(name="accvp", bufs=2) as accvp,
            tc.tile_pool(name="accTp", bufs=2) as accTp,
            tc.tile_pool(name="outTp", bufs=2) as outTp,
            tc.tile_pool(name="otqp", bufs=2) as otqp,
            tc.tile_pool(name="otp", bufs=2) as otp,
            tc.tile_pool(name="recp", bufs=2) as recp,
            tc.tile_pool(name="pst", bufs=2, space="PSUM") as pst,
            tc.tile_pool(name="ppv", bufs=2, space="PSUM") as ppv,
        ):
            _emit(nc, dram, wp, biasp, xtp, qkvp, ep, accp, accvp, accTp,
                  outTp, otqp, otp, recp, pst, ppv)

    nc.finalize()
    return nc


def _emit(nc, dram, wp, biasp, xtp, qkvp, ep, accp, accvp, accTp, outTp,
          otqp, otp, recp, pst, ppv):
    # ---------------- prologue loads, split across the two HWDGE rings ----
    # scalar ring: wq, wk, xq_a, xk_a       sync ring: wv, xv_a, biases
    ws = {}

    def load_w(wname, s, eng):
        D = 512 if s == "a" else 256
        w = wp.tile([128, CT, D], BF16, tag=f"{wname}_{s}", name=f"{wname}_{s}")
        eng.dma_start(
            out=w[:, :, :],
            in_=dram[f"{wname}_{s}"][:].rearrange("(c p) d -> p c d", p=128),
        )
        ws[f"{wname}_{s}"] = w

    xts = {}  # (slot, kind, ct) -> tile ; tags shared between slots

    def load_x(s, kind, eng):
        xr = dram[f"x{kind}_{s}"][:].rearrange("(c p) M -> p c M", p=128)
        for ct in range(CT):
            xt = xtp.tile([128, NTOK], BF16, tag=f"x{kind}{ct}", name=f"x{kind}{ct}")
            eng.dma_start(out=xt[:, :], in_=xr[:, ct])
            xts[(s, kind, ct)] = xt

    load_w("wq", "a", nc.scalar)
    load_w("wk", "a", nc.scalar)
    load_x("a", "q", nc.scalar)
    load_x("a", "k", nc.scalar)
    load_w("wv", "a", nc.sync)
    load_x("a", "v", nc.sync)
    load_w("wq", "b", nc.scalar)
    load_w("wk", "b", nc.scalar)
    load_w("wv", "b", nc.sync)

    bqk = {}
    bvb = {}
    for s, D in (("a", 512), ("b", 256)):
        nh = D // 128
        t = biasp.tile([128, 2, nh], F32, tag=f"bqk_{s}", name=f"bqk_{s}")
        nc.sync.dma_start(
            out=t[:, 0, :], in_=dram[f"bq_{s}"][:].rearrange("(j p) -> p j", p=128)
        )
        nc.sync.dma_start(
            out=t[:, 1, :], in_=dram[f"bk_{s}"][:].rearrange("(j p) -> p j", p=128)
        )
        bqk[s] = t
        bv = biasp.tile([128, D], BF16, tag=f"bvb_{s}", name=f"bvb_{s}")
        nc.sync.dma_start(
            out=bv[:, :], in_=dram[f"bv_{s}"][:].unsqueeze(0).to_broadcast([128, D])
        )
        bvb[s] = bv

    QT = {"a": qkvp.tile([128, 4, NTOK], BF16, tag="qt_a", name="qt_a"),
          "b": qkvp.tile([128, 2, NTOK], BF16, tag="qt_b", name="qt_b")}
    KT = {"a": qkvp.tile([128, 4, NTOK], BF16, tag="kt_a", name="kt_a"),
          "b": qkvp.tile([128, 2, NTOK], BF16, tag="kt_b", name="kt_b")}
    V = {"a": qkvp.tile([128, TT, 512], BF16, tag="v_a", name="v_a"),
         "b": qkvp.tile([128, TT, 256], BF16, tag="v_b", name="v_b")}

    # ---- projection sub-tasks: (emit_mms, emit_post) pairs.  The post
    # (PSUM->SBUF bias-add / copy on DVE) runs one injection slot later so
    # the DVE FIFO never blocks on un-run PE matmuls.
    def qk_sub(s, which, wname, dt, qc4):
        kind = "q" if which == 0 else "k"
        box = {}

        def mms():
            ps = ppv.tile([128, 512], F32, tag="psproj", name="psproj")
            for ct in range(CT):
                nc.tensor.matmul(
                    ps[:, :],
                    ws[f"{wname}_{s}"][:, ct, dt * 128 : (dt + 1) * 128],
                    xts[(s, kind, ct)][:, qc4 * 512 : (qc4 + 1) * 512],
                    start=(ct == 0),
                    stop=(ct == CT - 1),
                )
            box["ps"] = ps

        def post():
            dst = QT[s] if which == 0 else KT[s]
            nc.vector.tensor_scalar_add(
                dst[:, dt, qc4 * 512 : (qc4 + 1) * 512],
                box["ps"][:, :],
                bqk[s][:, which, dt : dt + 1],
            )
        return (mms, post)

    def v_sub(s, D, tt):
        box = {}

        def mms():
            ps = ppv.tile([128, 512], F32, tag="psproj", name="psproj")
            for ct in range(CT):
                nc.tensor.matmul(
                    ps[:, :D],
                    xts[(s, "v", ct)][:, tt * 128 : (tt + 1) * 128],
                    ws[f"wv_{s}"][:, ct, :],
                    start=(ct == 0),
                    stop=(ct == CT - 1),
                )
            box["ps"] = ps

        def post():
            nc.vector.tensor_copy(V[s][:, tt, :], box["ps"][:, :D])
        return (mms, post)

    def run_subs(subs):
        # emit a list of (mms, post) with posts deferred by one sub
        pending = None
        for mms, post in subs:
            mms()
            if pending is not None:
                pending()
            pending = post
        if pending is not None:
            pending()

    # slot A h0 Q/K emitted in prologue (before step 0), posts deferred
    run_subs([qk_sub("a", w, ("wq", "wk")[w], 0, qc4)
              for qc4 in range(NQC) for w in range(2)])

    # injection schedule: step -> list of (mms, post) or plain closures
    inject = {st: [] for st in range(26)}

    def add_qk(step0, s, h):
        subs = [qk_sub(s, w, ("wq", "wk")[w], h, qc4)
                for w in range(2) for qc4 in range(NQC)]
        for i, sub in enumerate(subs):  # 8 subs over 2 steps
            inject[step0 + i // 4].append(sub)

    def add_v(step0, s, D, nsteps):
        subs = [v_sub(s, D, tt) for tt in range(TT)]
        per = (len(subs) + nsteps - 1) // nsteps
        for i, sub in enumerate(subs):
            inject[step0 + i // per].append(sub)

    def load_sub(s, kind, eng):
        def emit():
            load_x(s, kind, eng)
        return (emit, None)

    add_v(0, "a", 512, 1)        # V_a fully in step 0 (AV_0 reads it step 1)
    add_qk(2, "a", 1)            # h1 by end of step 3 (used step 4)
    add_qk(4, "a", 2)            # h2 by end of step 5 (used step 8)
    add_qk(6, "a", 3)            # h3 by end of step 7 (used step 12)
    inject[1].append(load_sub("b", "v", nc.sync))    # xv tags free after step 0
    inject[8].append(load_sub("b", "q", nc.scalar))  # xq/xk tags free after 7
    inject[8].append(load_sub("b", "k", nc.sync))
    add_v(9, "b", 256, 4)        # steps 9-12 (AV_16 runs in step 17)
    add_qk(13, "b", 0)           # by end of step 14 (used step 16)
    add_qk(16, "b", 1)           # by end of step 17 (used step 20)

    # ---------------- the 24-unit software-pipelined stream ---------------
    units = [("a", h, qc) for h in range(4) for qc in range(NQC)] + \
            [("b", h, qc) for h in range(2) for qc in range(NQC)]

    state = {}

    def emit_scores_g(u, g, st_tile):
        s, h, qc = units[u]
        qsl = slice(qc * QCH, (qc + 1) * QCH)
        for j in range(2):
            kt = 2 * g + j
            nc.tensor.matmul(
                st_tile[:, j * QCH : (j + 1) * QCH],
                KT[s][:, h, kt * 128 : (kt + 1) * 128],
                QT[s][:, h, qsl],
                start=True,
                stop=True,
            )

    def emit_av_g(u, g):
        s, h, qc = units[u]
        stt = state[u]
        for j in range(2):
            kt = 2 * g + j
            nc.tensor.matmul(
                stt["pv"][:, :],
                V[s][:, kt, h * 128 : (h + 1) * 128],
                stt["E"][:, kt * QCH : (kt + 1) * QCH],
                start=(kt == 0),
                stop=(kt == TT - 1),
            )

    NSTEPS = len(units) + 2
    for step in range(NSTEPS):
        u_cur = step if step < len(units) else None
        u_prev = step - 1 if 1 <= step <= len(units) else None
        u_tail = step - 2 if step >= 2 else None

        if u_cur is not None:
            E = ep.tile([128, TT * QCH], BF16, tag="E", name="E")
            state[u_cur] = {"E": E}
        if u_prev is not None:
            pv = ppv.tile([128, QCH], F32, tag="pvav", name="pvav")
            state[u_prev]["pv"] = pv

        inj = list(inject.get(step, ()))
        ninj = len(inj)

        # interleaved PE stream: projections | attn@V(u-1) | scores(u).
        # Projection posts (DVE) are deferred one g-slot behind their MMs.
        pending_posts = []
        for g in range(8):
            lo = (g * ninj) // 8
            hi = ((g + 1) * ninj) // 8
            posts_due, pending_posts = pending_posts, []
            for mms, post in inj[lo:hi]:
                mms()
                if post is not None:
                    pending_posts.append(post)
            for p in posts_due:
                p()
            if u_prev is not None:
                emit_av_g(u_prev, g)
            if u_cur is not None:
                st_tile = pst.tile([128, 2 * QCH], F32, tag="st", name="st")
                emit_scores_g(u_cur, g, st_tile)
                nc.scalar.activation(
                    state[u_cur]["E"][:, g * 1024 : (g + 1) * 1024],
                    st_tile[:, :],
                    EXP,
                    scale=SCALE,
                )
        for p in pending_posts:
            p()

        # tail of unit u-2
        if u_tail is not None:
            ts, th, tqc = units[u_tail]
            stt = state[u_tail]
            accT = accTp.tile([128, NQC, 128], BF16, tag="accT", name="accT")
            nc.sync.dma_start_transpose(out=accT[:, :, :], in_=stt["acc"][:, 0:512])
            otq = otqp.tile([128, NQC, 128], BF16, tag="otq", name="otq")
            nc.sync.dma_start_transpose(out=otq[:, :, :], in_=stt["outT"][:, :])
            dcol4 = recp.tile([128, NQC], F32, tag="dcol4", name="dcol4")
            rec4 = recp.tile([128, NQC], F32, tag="rec4", name="rec4")
            nc.vector.reduce_sum(out=dcol4[:, :], in_=accT[:, :, :], axis=AXX)
            nc.vector.reciprocal(rec4[:, :], dcol4[:, :])
            ot = otp.tile([128, NQC, 128], BF16, tag="ot", name="ot")
            for j in range(NQC):
                nc.vector.scalar_tensor_tensor(
                    out=ot[:, j, :],
                    in0=otq[:, j, :],
                    scalar=rec4[:, j : j + 1],
                    in1=bvb[ts][:, th * 128 : (th + 1) * 128],
                    op0=MULT,
                    op1=ADD,
                )
            nc.sync.dma_start(
                out=dram[f"out_{ts}"][
                    tqc * QCH : (tqc + 1) * QCH, th * 128 : (th + 1) * 128
                ].rearrange("(j p) d -> p j d", p=128),
                in_=ot[:, :, :],
            )
            del state[u_tail]

        # tree-sum of unit u (gpsimd takes the contiguous L1 of k-tiles 0:8;
        # DVE takes k-tiles 8:16 and the merge levels) + cast of pv(u-1)
        if u_cur is not None:
            E = state[u_cur]["E"]
            acc_g = accp.tile([128, 2048], BF16, tag="acc_g", name="acc_g")
            acc_v = accvp.tile([128, 2048], BF16, tag="acc_v", name="acc_v")
            nc.gpsimd.tensor_add(acc_g[:, :], E[:, 0:2048], E[:, 2048:4096])
            nc.vector.tensor_add(acc_v[:, :], E[:, 4096:6144], E[:, 6144:8192])
            nc.vector.tensor_add(
                acc_v[:, 0:1024], acc_v[:, 0:1024], acc_v[:, 1024:2048]
            )
            if u_prev is not None:
                outT = outTp.tile([128, QCH], BF16, tag="outT", name="outT")
                nc.vector.tensor_copy(outT[:, :], state[u_prev]["pv"][:, :])
                state[u_prev]["outT"] = outT
            nc.vector.tensor_add(
                acc_g[:, 0:1024], acc_g[:, 0:1024], acc_g[:, 1024:2048]
            )
            nc.vector.tensor_add(
                acc_g[:, 0:1024], acc_g[:, 0:1024], acc_v[:, 0:1024]
            )
            nc.vector.tensor_add(
                acc_g[:, 0:512], acc_g[:, 0:512], acc_g[:, 512:1024]
            )
            state[u_cur]["acc"] = acc_g
        elif u_prev is not None:
            outT = outTp.tile([128, QCH], BF16, tag="outT", name="outT")
            nc.vector.tensor_copy(outT[:, :], state[u_prev]["pv"][:, :])
            state[u_prev]["outT"] = outT


_PROGRAM = None


def _get_program():
    global _PROGRAM
    if _PROGRAM is None:
        _PROGRAM = _build_program()
    return _PROGRAM


def kernel(query, key, value, Wq, bq, Wk, bk, Wv, bv):
    global LAST_RESULTS
    bf = ml_dtypes.bfloat16
    q = np.ascontiguousarray(
        np.asarray(query, np.float32).reshape(NBM, NTOK, DIM).transpose(0, 2, 1)
    ).astype(bf)
    k = np.ascontiguousarray(
        np.asarray(key, np.float32).reshape(NBM, NTOK, DIM).transpose(0, 2, 1)
    ).astype(bf)
    v = np.ascontiguousarray(
        np.asarray(value, np.float32).reshape(NBM, NTOK, DIM).transpose(0, 2, 1)
    ).astype(bf)
    WqT = np.ascontiguousarray(np.asarray(Wq, np.float32).T).astype(bf)
    WkT = np.ascontiguousarray(np.asarray(Wk, np.float32).T).astype(bf)
    WvT = np.ascontiguousarray(np.asarray(Wv, np.float32).T).astype(bf)
    bq = np.asarray(bq, np.float32)
    bk = np.asarray(bk, np.float32)
    bvb = np.asarray(bv, np.float32).astype(bf)

    in_maps = []
    for c in range(NCORES):
        bm_a = c
        bm_b = 8 + c // 2
        hs = (c % 2) * 256  # head-pair column offset for slot B
        in_maps.append(
            {
                "xq_a": q[bm_a], "xk_a": k[bm_a], "xv_a": v[bm_a],
                "xq_b": q[bm_b], "xk_b": k[bm_b], "xv_b": v[bm_b],
                "wq_a": WqT, "wk_a": WkT, "wv_a": WvT,
                "bq_a": bq, "bk_a": bk, "bv_a": bvb,
                "wq_b": np.ascontiguousarray(WqT[:, hs : hs + 256]),
                "wk_b": np.ascontiguousarray(WkT[:, hs : hs + 256]),
                "wv_b": np.ascontiguousarray(WvT[:, hs : hs + 256]),
                "bq_b": np.ascontiguousarray(bq[hs : hs + 256]),
                "bk_b": np.ascontiguousarray(bk[hs : hs + 256]),
                "bv_b": np.ascontiguousarray(bvb[hs : hs + 256]),
            }
        )

    nc = _get_program()
    res = run_bass_kernel_spmd(
        nc, in_maps, list(range(NCORES)), trace=TRACE, **TRACE_KWARGS
    )
    LAST_RESULTS = res

    out = np.empty((NBM, NTOK, DIM), np.float32)
    for c in range(NCORES):
        hs = (c % 2) * 256
        out[c] = np.asarray(res.results[c]["out_a"], np.float32)
        out[8 + c // 2][:, hs : hs + 256] = np.asarray(
            res.results[c]["out_b"], np.float32
        )
    return out.reshape(B, M, NTOK, DIM)
